# revision 22
# baseline (speedup 1.0000x reference)
"""Trainium2 Bass kernel: 4D convolution (kernel 3^4, stride 1, pad 1) + bias.

  out[b,o,t,d,h,w] = bias[o] +
      sum_{i,at,ad,ah,aw} x[b,i,t+at-1,d+ad-1,h+ah-1,w+aw-1] * W[o,i,at,ad,ah,aw]

Shapes: x [2,16,8,8,32,32], W [32,16,3,3,3,3], bias [32] -> out [2,32,8,8,32,32].

Distribution (8 cores): data-parallel over the 16 (b, t) output slices, 2
adjacent t's per core.  Each core gets a 4-plane t-slab (t0-1 .. t0+2, zero
padded at tensor edges) plus replicated (host-retransformed) weights, and
produces out[b, :, t0:t0+2].

Per-core algorithm ("(t,d)-banded implicit GEMM", 144 matmuls):
  * M columns pack BOTH output t and a d-pair: m = 64*u + 32*jd + o
    (u in {0,1} output t, jd in {0,1} within a d-pair group g in {0..3},
    o in 32 out-channels).
  * K partitions pack the full t-slab and a width-2 d_in window:
    p = 64*dq + 16*tq + i  (dq in {0,1} window row, tq in {0..3} slab
    plane, i in 16 in-channels).
  * The padded d_in range {-1..8} tiles EXACTLY into five width-2 windows
    XM[m'] = d_in {2m'-1, 2m'}, m' = 0..4.  Bank (g, hh) contracts window
    e=-1 (XM[g], ad in {0,1}-ish band) then e=+1 (XM[g+1]): the (jd, ad)
    cover is exact and disjoint:
      e=-1: jd=0 gets ad {0,1}, jd=1 gets ad {0}
      e=+1: jd=0 gets ad {2},   jd=1 gets ad {1,2}
  * Per pass (e, ah, aw): one matmul per (g, hh) bank, rhs free offsets
    give (ah, aw); at = tq - u and ad = e + dq - jd + 1 are baked into the
    18 g-independent banded weight tiles BW[9*e_idx + si] [128 K, 128 M]:
      BW[.][64dq+16tq+i, 64u+32jd+o] = W[o, i, tq-u, e+dq-jd+1, ah, aw]
    (zero when at or ad out of band; out-of-range d_in contributes zero
    via the zero-padded XM data).  8 PSUM banks = (g, hh), 18 matmuls
    each, N = 512 rows -> 144 matmuls of 512 rows total vs 216 for the
    plain d-banded scheme (the at-band packing is the 1.5x win).
  * bf16 operands (1 PE cycle/row, half the HBM bytes), fp32 PSUM.
  * Evict PSUM via ScalarE activation (identity + per-partition bias);
    each bank's 256 KB output is split across BOTH DMA queues (sync/HWDGE
    u=0 half, gpsimd/SWDGE u=1 half) since SBUF->HBM descriptors drain
    slowly (~46 B/ns measured) against the PE's SBUF read pressure.

HBM layouts are partition-major so every DMA descriptor is a long
contiguous run (SWDGE descriptor generation costs ~5ns each on GpSimd,
and sub-KB descriptors throttled the fp32 version):
  * xs: [5, 128, 1156] bf16 - XM[m'] is one contiguous 296 KB slice.
  * bw: [128, 18, 128] bf16 - k-range chunks are 256*Dk-byte runs.

The host-side input transforms (t-slab extraction + halo pad + window
split, banded weight layout, bias broadcast) are pure data-layout work
done in numpy inside kernel(); the hardware kernel consumes them as its
external inputs.
"""

import numpy as np
import ml_dtypes

BF16 = ml_dtypes.bfloat16

I_C, O_C = 16, 32
B_FULL, T_FULL, D, H, W = 2, 8, 8, 32, 32
HP, WP = H + 2, W + 2
PLANE = HP * WP  # 1156
N_CORES = 8
NBW = 18  # banded tiles: (e, ah, aw)
NXM = 5  # width-2 d_in windows

_NC_CACHE: list = []


def emit_conv(tc, y_d, xs_d, bw_d, bb_d):
    """Emit the per-core conv program into TileContext `tc`.

    y_d [2, 32, 8, 32, 32] f32 out; xs_d [5, 128, 1156] bf16 x windows;
    bw_d [128, 18, 128] bf16 banded weights (partition-major); bb_d [128] f32.
    """
    import concourse.mybir as mybir

    nc = tc.nc
    f32 = mybir.dt.float32
    bf16 = mybir.dt.bfloat16
    Ident = mybir.ActivationFunctionType.Identity

    with (
        tc.tile_pool(name="xpool", bufs=1) as xpool,
        tc.tile_pool(name="wpool", bufs=1) as wpool,
        tc.tile_pool(name="opool", bufs=4) as opool,
        tc.tile_pool(name="ppool", bufs=1, space="PSUM") as ppool,
    ):
        # ---- PSUM accumulators: 8 banks = (g, hh) ----
        acc = {}
        for g in range(4):
            for hh in range(2):
                acc[g, hh] = ppool.tile(
                    [128, 512], f32, name=f"acc{g}{hh}", tag=f"acc{g}{hh}"
                )

        # ---- warmup: keep the PE busy (and un-throttle HAM) during the
        # input-DMA lead-in.  Zero matmuls into bank 0; the first real
        # matmul there uses start=True, which discards these results.
        WZ = wpool.tile([128, 128], bf16, name="WZ")
        nc.vector.memset(WZ[:, :], 0.0)
        for _ in range(26):
            nc.tensor.matmul(
                out=acc[0, 0][:, 0:128],
                lhsT=WZ[:, :],
                rhs=WZ[:, :],
                start=True,
                stop=True,
            )

        BW = wpool.tile([128, NBW * 128], bf16, name="BW")
        BWv = BW.rearrange("p (k m) -> p k m", k=NBW)

        XM = xpool.tile([128, NXM * PLANE], bf16, name="XM")
        XMv = XM.rearrange("p (m h w) -> p m h w", m=NXM, h=HP, w=WP)

        # ---- input DMA schedule: everything on SWDGE (gpsimd), strict
        # first-use order (HWDGE drains small descriptors at only ~24 B/ns,
        # so routing anything first-use through it stalls the PE ramp).
        # XM tiles ship in two h-halves: hh=0 banks touch only h rows 0:18
        # (chunk A, free cols 0:612), hh=1 banks rows 16:34 (A+B).
        CA = 18 * WP  # 612

        def xm_a(m):
            nc.gpsimd.dma_start(out=XMv[:, m, 0:18, :], in_=xs_d[m, :, 0:CA])

        def xm_b(m):
            nc.gpsimd.dma_start(out=XMv[:, m, 18:34, :], in_=xs_d[m, :, CA:PLANE])

        # kickoff split across trigger engines (descriptor generation is
        # ~0.7us per dma_start and serializes per engine): sync/HWDGE takes
        # the small first weight chunk while gpsimd/SWDGE streams the rest
        # in first-use order.
        nc.gpsimd.dma_start(out=BWv[:, 0:2], in_=bw_d[:, 0:2])
        xm_a(0)
        nc.gpsimd.dma_start(out=BWv[:, 2:6], in_=bw_d[:, 2:6])
        nc.gpsimd.dma_start(out=BWv[:, 6:12], in_=bw_d[:, 6:12])
        xm_a(1)
        # late weight chunk rides the (otherwise idle) HWDGE queues in
        # parallel, freeing SWDGE bytes for the critical-path x chunks
        nc.sync.dma_start(out=BWv[:, 12:15], in_=bw_d[:, 12:15])
        nc.scalar.dma_start(out=BWv[:, 15:NBW], in_=bw_d[:, 15:NBW])
        # late chunks, merged into few triggers: h rows 18:34 of XM0+XM1,
        # then XM2..4 whole (src rearranged partition-major to match dst)
        xs_r = xs_d.rearrange("m p c -> p m c")
        nc.gpsimd.dma_start(out=XMv[:, 0:2, 18:34, :], in_=xs_r[:, 0:2, CA:PLANE])
        nc.gpsimd.dma_start(out=XMv[:, 2:5, :, :], in_=xs_r[:, 2:5, :])

        BB = wpool.tile([128, 1], f32, name="BB")
        nc.sync.dma_start(out=BB[:, :], in_=bb_d.rearrange("(p u) -> p u", u=1))

        # ---- main accumulation, bank-major: each bank's 18 passes run
        # consecutively so its eviction overlaps the remaining MM stream ----
        steps = [(ah, aw) for ah in range(3) for aw in range(3)]
        for g in range(4):
            for hh in range(2):
                for ei in range(2):
                    for si, (ah, aw) in enumerate(steps):
                        rhs = XMv[
                            :,
                            g + ei,
                            16 * hh + ah : 16 * hh + ah + 16,
                            aw : aw + W,
                        ]
                        nc.tensor.matmul(
                            out=acc[g, hh][:, :],
                            lhsT=BWv[:, 9 * ei + si, :],
                            rhs=rhs,
                            start=(ei == 0 and si == 0),
                            stop=(ei == 1 and si == len(steps) - 1),
                        )
                # evict this bank, split across three DMA queues (partition
                # slices keep the 1 KB HBM runs consecutive -> mergeable)
                ot = opool.tile([128, 512], bf16, name="ot", tag="ot")
                bank = 2 * g + hh
                nc.scalar.activation(
                    ot[:, :],
                    acc[g, hh][:, :],
                    Ident,
                    bias=BB[:, :],
                    scale=1.0,
                )
                for p0, p1, q in (
                    (0, 32, nc.sync),
                    (32, 64, nc.scalar),
                    (64, 128, nc.gpsimd),
                ):
                    q.dma_start(out=y_d[bank, p0:p1, :], in_=ot[p0:p1, :])


def build_nc():
    if _NC_CACHE:
        return _NC_CACHE[0]
    import concourse.bacc as bacc
    import concourse.mybir as mybir
    from concourse.tile import TileContext

    f32 = mybir.dt.float32
    bf16 = mybir.dt.bfloat16
    nc = bacc.Bacc("TRN2", target_bir_lowering=False, debug=False, num_devices=N_CORES)
    xs_d = nc.dram_tensor("xs", [NXM, 128, PLANE], bf16, kind="ExternalInput").ap()
    bw_d = nc.dram_tensor("bw", [128, NBW, 128], bf16, kind="ExternalInput").ap()
    bb_d = nc.dram_tensor("bb", [128], f32, kind="ExternalInput").ap()
    # raw bank layout: y[2g+hh, 64u+32jd+o, 16h+w]; host un-permutes.
    # Per-partition 1 KB runs are consecutive in HBM, so the DMA engines can
    # merge a bank eviction into a few large descriptors.
    y_d = nc.dram_tensor("y", [8, 128, 512], bf16, kind="ExternalOutput").ap()
    with TileContext(nc) as tc:
        emit_conv(tc, y_d, xs_d, bw_d, bb_d)
    nc.compile()
    _NC_CACHE.append(nc)
    return nc


def build_banded_weights(weight):
    """W [32,16,3,3,3,3] -> bw [128, 18, 128] bf16 banded tiles, p-major.

    Tile k = 9*e_idx + step(ah, aw):
      bw[64dq+16tq+i, k, 64u+32jd+o] = W[o, i, at=tq-u, ad=e+dq-jd+1, ah, aw]
    for at, ad in band, else 0.
    """
    bw = np.zeros((NBW, 128, 128), dtype=np.float32)
    steps = [(ah, aw) for ah in range(3) for aw in range(3)]
    for ei, e in enumerate((-1, 1)):
        for si, (ah, aw) in enumerate(steps):
            k = 9 * ei + si
            for dq in range(2):
                for tq in range(4):
                    for u in range(2):
                        at = tq - u
                        if not 0 <= at <= 2:
                            continue
                        for jd in range(2):
                            ad = e + dq - jd + 1
                            if not 0 <= ad <= 2:
                                continue
                            p0 = 64 * dq + 16 * tq
                            m0 = 64 * u + 32 * jd
                            bw[k, p0 : p0 + 16, m0 : m0 + 32] = weight[
                                :, :, at, ad, ah, aw
                            ].T
    # partition-major for contiguous per-partition DMA runs
    return np.ascontiguousarray(bw.transpose(1, 0, 2).astype(BF16))


def shard_inputs(x, weight, bias):
    """Full inputs -> per-core in_maps (x window tiles, banded weights, bias)."""
    x = np.ascontiguousarray(np.asarray(x, dtype=np.float32))
    weight = np.ascontiguousarray(np.asarray(weight, dtype=np.float32))
    bias = np.ascontiguousarray(np.asarray(bias, dtype=np.float32))

    bw = build_banded_weights(weight)
    bb = np.ascontiguousarray(np.tile(bias, 4))  # column m = 64u+32jd+o -> bias[o]

    in_maps = []
    for c in range(N_CORES):
        b = c // 4
        t0 = 2 * (c % 4)
        slab = np.zeros((I_C, 4, D, H, W), dtype=np.float32)
        lo, hi = t0 - 1, t0 + 3
        slo, shi = max(lo, 0), min(hi, T_FULL)
        slab[:, slo - lo : shi - lo] = x[b, :, slo:shi]
        # xs[dpad, tq, i, h+halo, w+halo]; dpad = d_in + 1 in {0..9};
        # contiguous width-2 windows XM[m'] = xs[2m':2m'+2]
        xs = np.zeros((10, 4, I_C, HP, WP), dtype=BF16)
        xs[1:9, :, :, 1 : 1 + H, 1 : 1 + W] = slab.transpose(2, 1, 0, 3, 4)
        in_maps.append(
            {"xs": np.ascontiguousarray(xs.reshape(NXM, 128, PLANE)), "bw": bw, "bb": bb}
        )
    return in_maps


def unshard_outputs(results):
    out = np.empty((B_FULL, O_C, T_FULL, D, H, W), dtype=np.float32)
    for c in range(N_CORES):
        b = c // 4
        t0 = 2 * (c % 4)
        y = np.asarray(results[c]["y"], dtype=np.float32)
        # [8 banks=(g,hh), 128=(u,jd,o), 512=(h,w)] -> [u, o, d=(g,jd), h, w]
        y = y.reshape(4, 2, 2, 2, O_C, 16, W).transpose(2, 4, 0, 3, 1, 5, 6)
        y = y.reshape(2, O_C, D, H, W)
        out[b, :, t0] = y[0]
        out[b, :, t0 + 1] = y[1]
    return out


def run(inputs, trace=False, **kwargs):
    from concourse.bass_utils import run_bass_kernel_spmd

    nc = build_nc()
    in_maps = shard_inputs(inputs["x"], inputs["weight"], inputs["bias"])
    res = run_bass_kernel_spmd(
        nc, in_maps, core_ids=list(range(N_CORES)), trace=trace, **kwargs
    )
    return unshard_outputs(res.results), res


def kernel(x, weight, bias):
    out, _ = run({"x": x, "weight": weight, "bias": bias})
    return out
